# revision 1
# baseline (speedup 1.0000x reference)
"""CRF negative log-likelihood on 8 Trainium2 NeuronCores.

Strategy (data-parallel over batch, 16 sequences per core):
  - The log-partition function runs in *linear space*: with E = exp(trans)
    and Mem = exp(emissions) (bf16),
        fwd:  A_{t+1} = (E^T A_t) . Mem[t+1]
        bwd:  B_{t-1} = (E B_t)   . Mem[t-1]
    Each core runs BOTH chains concurrently (fwd from t=0, bwd from t=T-1)
    and they meet in the middle:  Z = sum_{c,c'} A_m[c] E[c,c'] B_{m+1}[c'].
    This halves the serial-latency-bound round count; the two independent
    chains also hide each other's PE->DVE->PE latency.
  - Each chain step is 4 accumulating 128x128xN matmuls (the 256x256
    transition matrix in chunks, stationary bf16 operand, FWL) plus one
    elementwise PSUM*Mem multiply on the vector engine.
  - Every K_REB rounds (and at each chain's last round) the state is
    rescaled by ~1/P[0, b] (bf16 reciprocal); the exact log of the applied
    scale is recovered at the end via one Ln over the stored reciprocals
    (with a 2^64 pre-scale to stay inside the Ln table's accurate range).
  - Gold (numerator) path score is computed on-device from one-hot masks:
        em part:    reduce_t (em * onehot(tags))
        trans part: W = trans^T-matmul(onehot(tags_{t+1})),
                    reduce (W * onehot(tags_t))
        start/end:  tiny one-hot matmuls
  - Output per core: [nll(16) | logZ(16) | gold(16) | debug]; the host
    averages the 128 per-sequence NLL values.

The host only shards inputs, re-lays-out arrays for DMA efficiency
(pure transposes of the same values), and averages at the end.
"""

import math
import os
from contextlib import ExitStack

import numpy as np

import concourse.bass as bass
import concourse.bacc as bacc
import concourse.mybir as mybir
import concourse.tile as tile
from concourse.bass_utils import run_bass_kernel_spmd

# Problem shape (fixed by the task).
B, T, C = 128, 512, 256
NCORES = 8
BL = B // NCORES            # sequences per core (16)
NCH = C // 128              # partition chunks of the tag dimension (2)

K_REB = int(os.environ.get("CRF_KREB", "12"))     # rescale period (rounds)
T_RUN = int(os.environ.get("CRF_T", str(T)))     # time steps actually run

FP32 = mybir.dt.float32
BF16 = mybir.dt.bfloat16
I32 = mybir.dt.int32
AF = mybir.ActivationFunctionType
OP = mybir.AluOpType
AX = mybir.AxisListType

_LAST_EXEC_NS = None
_CACHE = {}


def _build_nc():
    nc = bacc.Bacc()
    em_d = nc.declare_dram_parameter("em", [C, T, BL], FP32, isOutput=False)
    tags_d = nc.declare_dram_parameter("tags", [128, T * BL], FP32, isOutput=False)
    tr_d = nc.declare_dram_parameter("trans", [C, C], FP32, isOutput=False)
    trT_d = nc.declare_dram_parameter("transT", [C, C], FP32, isOutput=False)
    st_d = nc.declare_dram_parameter("start2", [128, NCH], FP32, isOutput=False)
    en_d = nc.declare_dram_parameter("end2", [128, NCH], FP32, isOutput=False)
    out_d = nc.declare_dram_parameter("out", [6 * BL], FP32, isOutput=True)

    with tile.TileContext(nc) as tc:
        with ExitStack() as ctx:
            _body(ctx, tc, nc, em_d, tags_d, tr_d, trT_d, st_d, en_d, out_d)
    nc.finalize()
    return nc


def _body(ctx, tc, nc, em_d, tags_d, tr_d, trT_d, st_d, en_d, out_d):
    Trun = T_RUN
    assert Trun >= 4
    F = T * BL                      # free size per chunk (8192)
    FB = NCH * BL                   # chain-state free size (32)
    HM = Trun // 2
    NF = HM - 1                     # fwd rounds (A_{NF} covers em[0..HM-1])
    NB = Trun - 1 - HM              # bwd rounds (B covers em[HM..Trun-1])
    reb_f = sorted({r for r in range(1, NF + 1) if r % K_REB == 0} |
                   ({NF} if NF >= 1 else set()))
    reb_b = sorted({r for r in range(1, NB + 1) if r % K_REB == 0} |
                   ({NB} if NB >= 1 else set()))
    n_slots = len(reb_f) + len(reb_b)

    sing = ctx.enter_context(tc.tile_pool(name="sing", bufs=1))
    stg = ctx.enter_context(tc.tile_pool(name="stg", bufs=2))
    apool = ctx.enter_context(tc.tile_pool(name="apool", bufs=4))
    wcp = ctx.enter_context(tc.tile_pool(name="wcp", bufs=2))
    gsc = ctx.enter_context(tc.tile_pool(name="gsc", bufs=4))
    # PSUM: 8 banks total -> P:4, W:2, psb:1, misc:1
    pp = ctx.enter_context(tc.tile_pool(name="pp", bufs=4, space="PSUM"))
    pw = ctx.enter_context(tc.tile_pool(name="pw", bufs=2, space="PSUM"))
    pb = ctx.enter_context(tc.tile_pool(name="pb", bufs=1, space="PSUM"))
    pm = ctx.enter_context(tc.tile_pool(name="pm", bufs=1, space="PSUM"))

    # ---- persistent SBUF tensors ----
    em_t = sing.tile([128, NCH * F], FP32, tag="em")       # f = j*F + t*BL + b
    mem_t = sing.tile([128, NCH * F], BF16, tag="mem")
    oh_t = sing.tile([128, NCH * F], BF16, tag="oh")
    tags_t = sing.tile([128, F], FP32, tag="tags")
    e_t = sing.tile([128, NCH * C], BF16, tag="E")         # exp(trans),  f=i*C+c'
    e2_t = sing.tile([128, NCH * C], BF16, tag="E2")       # exp(trans^T), f=i*C+c
    trT_t = sing.tile([128, NCH * C], BF16, tag="trT")     # raw trans^T
    stE_t = sing.tile([128, NCH], FP32, tag="stE")
    stR_t = sing.tile([128, NCH], BF16, tag="stR")
    enEf_t = sing.tile([128, NCH], FP32, tag="enEf")
    enR_t = sing.tile([128, NCH], BF16, tag="enR")
    cval_t = sing.tile([128, NCH], I32, tag="cval")
    cvalf_t = sing.tile([128, NCH], FP32, tag="cvalf")
    ones_c = sing.tile([128, 1], FP32, tag="onesc")
    ones_cb = sing.tile([128, 1], BF16, tag="onescb")
    ones_r = sing.tile([1, 128], BF16, tag="onesr")
    dbuf_t = sing.tile([1, max(n_slots, 1) * FB], BF16, tag="dbuf")
    logd_t = sing.tile([1, max(n_slots, 1) * FB], FP32, tag="logd")
    r_t = sing.tile([1, BL], FP32, tag="R")
    vmid_t = sing.tile([128, FB], FP32, tag="vmid")
    fin_t = sing.tile([1, BL], FP32, tag="fin")
    finl_t = sing.tile([1, BL], FP32, tag="finl")
    logz_t = sing.tile([1, BL], FP32, tag="logz")
    emdot_t = sing.tile([1, BL], FP32, tag="emdot")
    trdot_t = sing.tile([1, BL], FP32, tag="trdot")
    se_t = sing.tile([1, BL], FP32, tag="se")
    gold_t = sing.tile([1, BL], FP32, tag="gold")
    out_t = sing.tile([1, 6 * BL], FP32, tag="outt")

    emv = em_t[:].rearrange("p (j t b) -> p j t b", j=NCH, t=T, b=BL)
    memv = mem_t[:].rearrange("p (j t b) -> p j t b", j=NCH, t=T, b=BL)
    ohv = oh_t[:].rearrange("p (j t b) -> p j t b", j=NCH, t=T, b=BL)
    emdv = em_d[:].rearrange("(j p) t b -> p j t b", p=128)

    # ---- small input DMAs first (their consumers must not stall) ----
    nc.sync.dma_start(out=tags_t[:], in_=tags_d[:])
    trst = stg.tile([128, C], FP32, tag="trstage")
    trst2 = stg.tile([128, C], FP32, tag="trstage")
    for i in range(NCH):
        s = trst if i == 0 else trst2
        nc.sync.dma_start(out=s[:], in_=tr_d[i * 128:(i + 1) * 128, :])
        nc.scalar.activation(e_t[:, i * C:(i + 1) * C], s[:], AF.Exp)
    trstT = stg.tile([128, C], FP32, tag="trstageT")
    trstT2 = stg.tile([128, C], FP32, tag="trstageT")
    for k in range(NCH):
        s = trstT if k == 0 else trstT2
        nc.sync.dma_start(out=s[:], in_=trT_d[k * 128:(k + 1) * 128, :])
        nc.vector.tensor_copy(trT_t[:, k * C:(k + 1) * C], s[:])
        nc.scalar.activation(e2_t[:, k * C:(k + 1) * C], s[:], AF.Exp)
    stst = stg.tile([128, NCH], FP32, tag="sestage")
    enst = stg.tile([128, NCH], FP32, tag="sestage")
    nc.sync.dma_start(out=stst[:], in_=st_d[:])
    nc.sync.dma_start(out=enst[:], in_=en_d[:])
    nc.scalar.activation(stE_t[:], stst[:], AF.Exp)
    nc.vector.tensor_copy(stR_t[:], stst[:])
    nc.scalar.activation(enEf_t[:], enst[:], AF.Exp)
    nc.vector.tensor_copy(enR_t[:], enst[:])

    # ---- constants ----
    nc.gpsimd.memset(ones_c[:], 1.0)
    nc.gpsimd.memset(ones_cb[:], 1.0)
    nc.gpsimd.memset(ones_r[:], 1.0)
    for j in range(NCH):
        nc.gpsimd.iota(cval_t[:, j:j + 1], pattern=[[0, 1]], base=j * 128,
                       channel_multiplier=1)
    nc.vector.tensor_copy(cvalf_t[:], cval_t[:])

    # ---- one-hot of tags (DVE; only needs tags, runs before em arrives) ----
    for j in range(NCH):
        nc.vector.tensor_scalar(
            out=oh_t[:, j * F:(j + 1) * F], in0=tags_t[:],
            scalar1=cvalf_t[:, j:j + 1], scalar2=None, op0=OP.is_equal)

    # ---- emission DMAs + exp, alternating ends so both chains start early ----
    TBLK = 64
    nblk = (Trun + TBLK - 1) // TBLK
    order = []
    lo, hi = 0, nblk - 1
    while lo <= hi:
        order.append(lo)
        if hi != lo:
            order.append(hi)
        lo, hi = lo + 1, hi - 1
    for blk in order:
        t0, t1 = blk * TBLK, min((blk + 1) * TBLK, Trun)
        for j in range(NCH):
            nc.sync.dma_start(out=emv[:, j, t0:t1, :], in_=emdv[:, j, t0:t1, :])
    for blk in order:
        t0, t1 = blk * TBLK, min((blk + 1) * TBLK, Trun)
        for j in range(NCH):
            nc.scalar.activation(memv[:, j, t0:t1, :], emv[:, j, t0:t1, :],
                                 AF.Exp)

    # ---- chain inits ----
    # fwd: A_0 = exp(start) * Mem[0];  bwd: B_{T-1} = exp(end) * Mem[T-1]
    state = {}
    for name, t0, scal in (("f", 0, stE_t), ("b", Trun - 1, enEf_t)):
        a0 = apool.tile([128, FB], BF16, tag=f"A{name}")
        for j in range(NCH):
            nc.vector.tensor_scalar(
                out=a0[:, j * BL:(j + 1) * BL],
                in0=memv[:, j, t0, :],
                scalar1=scal[:, j:j + 1], scalar2=None, op0=OP.mult)
        state[name] = a0

    # ---- gold work units: short stages spread across rounds so no engine
    # stream ever hosts a long or cross-engine-blocking gold op.
    # W unit: W = trans^T-matmul(onehot_{t+1}) (PE) -> *onehot_t (DVE,
    # PSUM-direct) -> partition-sum ones-matmul (PE) -> per-seq partial
    # reduce (DVE).  em unit: same without the leading matmul. ----
    WT = 32
    ttot = Trun - 1
    nwu = (ttot + WT - 1) // WT
    neu = (Trun + WT - 1) // WT
    gpart_w = sing.tile([1, max(nwu, 1) * BL], FP32, tag="gpw")
    gpart_e = sing.tile([1, max(neu, 1) * BL], FP32, tag="gpe")

    def w_unit(k):
        ts0 = k * WT
        cnt = min(WT, ttot - ts0)
        st = {}

        def mms(i, w):
            first = None
            for kk in range(NCH):
                ret = nc.tensor.matmul(
                    w[:, :cnt * BL],
                    trT_t[:, kk * C + i * 128:kk * C + (i + 1) * 128],
                    ohv[:, kk, ts0 + 1:ts0 + 1 + cnt, :],
                    start=(kk == 0), stop=(kk == NCH - 1))
                first = first or ret
            return first

        def s1():
            w = pw.tile([128, WT * BL], FP32, tag="W")
            first = mms(0, w)
            st["w0"] = w
            return first

        def s2():
            wc = wcp.tile([128, WT * BL], BF16, tag="Wc")
            first = nc.scalar.copy(wc[:, :cnt * BL], st["w0"][:, :cnt * BL])
            st["c0"] = wc
            w = pw.tile([128, WT * BL], FP32, tag="W")
            mms(1, w)
            st["w1"] = w
            return first

        def s3():
            wc = wcp.tile([128, WT * BL], BF16, tag="Wc")
            first = nc.scalar.copy(wc[:, :cnt * BL], st["w1"][:, :cnt * BL])
            st["c1"] = wc
            vw = gsc.tile([128, WT * BL], BF16, tag="VW")
            nc.vector.tensor_tensor(
                out=vw[:, :cnt * BL], in0=st["c0"][:, :cnt * BL],
                in1=ohv[:, 0, ts0:ts0 + cnt, :], op=OP.mult)
            st["v0"] = vw
            vw1 = gsc.tile([128, WT * BL], BF16, tag="VW")
            nc.vector.tensor_tensor(
                out=vw1[:, :cnt * BL], in0=st["c1"][:, :cnt * BL],
                in1=ohv[:, 1, ts0:ts0 + cnt, :], op=OP.mult)
            st["v1"] = vw1
            return first

        def s4():
            es = pm.tile([1, WT * BL], FP32, tag="misc")
            first = None
            for i, v in enumerate((st["v0"], st["v1"])):
                ret = nc.tensor.matmul(es[0:1, :cnt * BL], ones_cb[:],
                                       v[:, :cnt * BL],
                                       start=(i == 0), stop=(i == 1))
                first = first or ret
            st["es"] = es
            return first

        def s5():
            esv = st["es"][0:1, :cnt * BL].rearrange(
                "p (t b) -> p b t", t=cnt, b=BL)
            return nc.vector.tensor_reduce(
                out=gpart_w[0:1, k * BL:(k + 1) * BL], in_=esv,
                axis=AX.X, op=OP.add)

        return [s1, s2, s3, s4, s5]

    def em_unit(k):
        ts0 = k * WT
        cnt = min(WT, Trun - ts0)
        st = {}

        def vem(j):
            v = gsc.tile([128, WT * BL], BF16, tag="Vem")
            first = nc.vector.tensor_tensor(
                out=v[:, :cnt * BL], in0=emv[:, j, ts0:ts0 + cnt, :],
                in1=ohv[:, j, ts0:ts0 + cnt, :], op=OP.mult)
            st[j] = v
            return first

        def s3():
            es = pm.tile([1, WT * BL], FP32, tag="misc")
            first = None
            for j in range(NCH):
                ret = nc.tensor.matmul(es[0:1, :cnt * BL], ones_cb[:],
                                       st[j][:, :cnt * BL],
                                       start=(j == 0), stop=(j == NCH - 1))
                first = first or ret
            st["es"] = es
            return first

        def s4():
            esv = st["es"][0:1, :cnt * BL].rearrange(
                "p (t b) -> p b t", t=cnt, b=BL)
            return nc.vector.tensor_reduce(
                out=gpart_e[0:1, k * BL:(k + 1) * BL], in_=esv,
                axis=AX.X, op=OP.add)

        return [lambda: vem(0), lambda: vem(1), s3, s4]

    def chain_step(name, lhsT_t, t, do_reb, slot):
        a = state[name]
        p = pp.tile([128, FB], FP32, tag="P")
        for j in range(NCH):
            for i in range(NCH):
                nc.tensor.matmul(
                    p[:, j * BL:(j + 1) * BL],
                    lhsT_t[:, (i * NCH + j) * 128:(i * NCH + j + 1) * 128],
                    a[:, i * BL:(i + 1) * BL],
                    start=(i == 0), stop=(i == NCH - 1))
        an = apool.tile([128, FB], BF16, tag=f"A{name}")
        pv = p[:].rearrange("p (j b) -> p j b", j=NCH)
        msl = memv[:, :, t, :]
        anv = an[:].rearrange("p (j b) -> p j b", j=NCH)
        ret = None
        if not do_reb:
            ret = nc.vector.tensor_tensor(out=anv, in0=pv, in1=msl, op=OP.mult)
        else:
            dcol = slot * FB
            with nc.allow_low_precision("rescale is exactly compensated"):
                for j in range(NCH):
                    nc.vector.reciprocal(
                        out=dbuf_t[0:1, dcol + j * BL:dcol + (j + 1) * BL],
                        in_=p[0:1, 0:BL])
            psb = pb.tile([128, FB], FP32, tag="psb")
            nc.tensor.matmul(psb[:], ones_r[:],
                             dbuf_t[0:1, dcol:dcol + FB],
                             start=True, stop=True)
            tmp = apool.tile([128, FB], BF16, tag=f"tmp{name}")
            tmpv = tmp[:].rearrange("p (j b) -> p j b", j=NCH)
            nc.vector.tensor_tensor(out=tmpv, in0=pv, in1=msl, op=OP.mult)
            ret = nc.vector.tensor_tensor(out=an[:], in0=tmp[:], in1=psb[:],
                                          op=OP.mult)
        state[name] = an
        return ret

    # ---- main loop: both chains advance once per round; gold stages are
    # scheduled at fixed rounds, 3 rounds apart within a unit ----
    units = []
    for k in range(max(nwu, neu)):
        if k < nwu:
            units.append(w_unit(k))
        if k < neu:
            units.append(em_unit(k))
    GSTART, GSTRIDE, SSTRIDE = 48, 6, 3
    sched = {}
    for uix, stages in enumerate(units):
        for six, fn in enumerate(stages):
            r0 = GSTART + uix * GSTRIDE + six * SSTRIDE
            sched.setdefault(r0, []).append((uix, six, fn))

    slot = 0
    nrounds = max(NF, NB)
    for r in range(1, nrounds + 1):
        last_chain = None
        for name, lhsT_t, nsteps, rebs, tfun in (
                ("f", e_t, NF, reb_f, lambda rr: rr),
                ("b", e2_t, NB, reb_b, lambda rr: Trun - 1 - rr)):
            if r > nsteps:
                continue
            do_reb = r in rebs
            ci = chain_step(name, lhsT_t, tfun(r), do_reb, slot)
            last_chain = ci or last_chain
            if do_reb:
                slot += 1
        for uix, six, fn in sorted(sched.get(r, [])):
            fn()
    for r in sorted(k for k in sched if k > nrounds):
        for uix, six, fn in sorted(sched[r]):
            fn()

    # ---- merge in the middle: Z = sum A_m E B_{m+1} ----
    u_ps = pp.tile([128, FB], FP32, tag="P")
    af, ab = state["f"], state["b"]
    for j in range(NCH):
        for i in range(NCH):
            nc.tensor.matmul(
                u_ps[:, j * BL:(j + 1) * BL],
                e_t[:, (i * NCH + j) * 128:(i * NCH + j + 1) * 128],
                af[:, i * BL:(i + 1) * BL],
                start=(i == 0), stop=(i == NCH - 1))
    nc.vector.tensor_tensor(out=vmid_t[:], in0=u_ps[:], in1=ab[:], op=OP.mult)
    z_ps = pm.tile([1, FB], FP32, tag="misc")
    nc.tensor.matmul(z_ps[0:1, :], ones_c[:], vmid_t[:], start=True, stop=True)
    zsb_t = sing.tile([1, FB], FP32, tag="zsb")
    nc.scalar.copy(zsb_t[:], z_ps[0:1, :])
    nc.vector.tensor_add(fin_t[:], zsb_t[0:1, 0:BL], zsb_t[0:1, BL:2 * BL])
    nc.scalar.activation(finl_t[:], fin_t[:], AF.Ln)
    if n_slots > 0:
        nc.scalar.activation(logd_t[:], dbuf_t[:], AF.Ln,
                             scale=float(2.0 ** 64))
        ldv = logd_t[0:1, :].rearrange("p (s j b) -> p b j s",
                                       s=n_slots, j=NCH, b=BL)
        nc.vector.tensor_reduce(out=r_t[0:1, :], in_=ldv[:, :, 0, :],
                                axis=AX.X, op=OP.add)
        nc.vector.tensor_sub(logz_t[:], finl_t[:], r_t[:])
        corr = float(n_slots * 64.0 * math.log(2.0))
        nc.vector.tensor_scalar(out=logz_t[:], in0=logz_t[:], scalar1=corr,
                                scalar2=None, op0=OP.add)
    else:
        nc.vector.tensor_copy(logz_t[:], finl_t[:])

    # ---- gold: combine the per-unit partials ----
    gwv = gpart_w[0:1, :].rearrange("p (w b) -> p b w", w=nwu, b=BL)
    nc.vector.tensor_reduce(out=trdot_t[:], in_=gwv, axis=AX.X, op=OP.add)
    gev = gpart_e[0:1, :].rearrange("p (w b) -> p b w", w=neu, b=BL)
    nc.vector.tensor_reduce(out=emdot_t[:], in_=gev, axis=AX.X, op=OP.add)

    # ---- gold: start/end part ----
    se_ps = pm.tile([1, BL], FP32, tag="misc")
    for j in range(NCH):
        nc.tensor.matmul(se_ps[0:1, :], stR_t[:, j:j + 1], ohv[:, j, 0, :],
                         start=(j == 0), stop=False)
    for j in range(NCH):
        nc.tensor.matmul(se_ps[0:1, :], enR_t[:, j:j + 1],
                         ohv[:, j, Trun - 1, :],
                         start=False, stop=(j == NCH - 1))
    nc.scalar.copy(se_t[:], se_ps[0:1, :])

    # ---- assemble output ----
    nc.vector.tensor_add(gold_t[:], emdot_t[:], trdot_t[:])
    nc.vector.tensor_add(gold_t[:], gold_t[:], se_t[:])
    nc.vector.tensor_sub(out_t[0:1, 0:BL], logz_t[:], gold_t[:])
    nc.vector.tensor_copy(out_t[0:1, BL:2 * BL], logz_t[:])
    nc.vector.tensor_copy(out_t[0:1, 2 * BL:3 * BL], gold_t[:])
    nc.vector.tensor_copy(out_t[0:1, 3 * BL:4 * BL], fin_t[:])
    nc.vector.tensor_copy(out_t[0:1, 4 * BL:5 * BL], af[0:1, 0:BL])
    nc.vector.tensor_copy(out_t[0:1, 5 * BL:6 * BL], ab[0:1, 0:BL])
    nc.sync.dma_start(out=out_d[:].rearrange("(o f) -> o f", o=1),
                      in_=out_t[0:1, :])


def _host_reference(emissions, tags, mask, transitions, start_transitions,
                    end_transitions):
    """Exact numpy fallback (only used if mask is not all ones)."""
    em = emissions.astype(np.float64)
    tr = transitions.astype(np.float64)
    st = start_transitions.astype(np.float64)
    en = end_transitions.astype(np.float64)
    m = mask.astype(bool)
    Bq, Tq, Cq = em.shape
    alpha = st[None, :] + em[:, 0]
    for t in range(1, Tq):
        s = alpha[:, :, None] + tr[None]
        mx = s.max(1)
        na = mx + np.log(np.exp(s - mx[:, None, :]).sum(1)) + em[:, t]
        alpha = np.where(m[:, t][:, None], na, alpha)
    z = alpha + en[None, :]
    mx = z.max(1)
    logZ = mx + np.log(np.exp(z - mx[:, None]).sum(1))
    mf = m.astype(np.float64)
    bidx = np.arange(Bq)
    em_sc = em[bidx[:, None], np.arange(Tq)[None, :], tags]
    tr_sc = tr[tags[:, :-1], tags[:, 1:]]
    score = st[tags[:, 0]] + em_sc[:, 0]
    score = score + ((tr_sc + em_sc[:, 1:]) * mf[:, 1:]).sum(1)
    lengths = m.sum(1).astype(np.int64) - 1
    last = tags[bidx, lengths]
    score = score + en[last]
    return np.float32((logZ - score).mean())


def kernel(emissions, tags, mask, transitions, start_transitions,
           end_transitions):
    global _LAST_EXEC_NS
    emissions = np.ascontiguousarray(np.asarray(emissions, dtype=np.float32))
    tags_i = np.asarray(tags).astype(np.int64)
    mask_np = np.asarray(mask).astype(bool)
    trans = np.ascontiguousarray(np.asarray(transitions, dtype=np.float32))
    start = np.asarray(start_transitions, dtype=np.float32)
    end = np.asarray(end_transitions, dtype=np.float32)

    if not mask_np.all():
        return _host_reference(emissions, tags_i, mask_np, trans, start, end)

    transT = np.ascontiguousarray(trans.T)
    start2 = np.ascontiguousarray(start.reshape(NCH, 128).T)
    end2 = np.ascontiguousarray(end.reshape(NCH, 128).T)
    tags32 = tags_i.astype(np.int32)

    in_maps = []
    for i in range(NCORES):
        sh = emissions[i * BL:(i + 1) * BL]                    # [BL, T, C]
        emT = np.ascontiguousarray(sh.transpose(2, 1, 0))      # [C, T, BL]
        tg1 = np.ascontiguousarray(
            tags32[i * BL:(i + 1) * BL].T).reshape(-1).astype(np.float32)
        tg = np.ascontiguousarray(np.broadcast_to(tg1[None, :], (128, T * BL)))
        in_maps.append({
            "em": emT, "tags": tg, "trans": trans, "transT": transT,
            "start2": start2, "end2": end2,
        })

    if "nc" not in _CACHE:
        _CACHE["nc"] = _build_nc()
    nc = _CACHE["nc"]

    trace = bool(int(os.environ.get("CRF_TRACE", "0")))
    try:
        res = run_bass_kernel_spmd(nc, in_maps, list(range(NCORES)), trace=trace)
    except Exception:
        if not trace:
            raise
        res = run_bass_kernel_spmd(nc, in_maps, list(range(NCORES)))
    _LAST_EXEC_NS = getattr(res, "exec_time_ns", None)

    _CACHE["last_results"] = [np.asarray(res.results[i]["out"])
                              for i in range(NCORES)]
    nll = np.concatenate([np.asarray(res.results[i]["out"])[0:BL]
                          for i in range(NCORES)])
    return np.float32(nll.mean())

